# revision 1
# baseline (speedup 1.0000x reference)
"""Bass/Tile Trainium2 kernel for nn_AttentionSampling.

Problem: out = q + attention_downsampling(LN(q), LN(k), LN(v), factor=4)
  B=4, Sq=2048, Skv=8192, D=1024. Per query token s:
    w_f   = dot(LN(q)[s], LN(k)[4s+f])          f in 0..3  (no softmax)
    out[s] = q[s] + sum_f w_f * LN(v)[4s+f]

Key algebraic folding (valid for ln_weight==1, ln_bias==0, which is what
setup_inputs produces; a numpy fallback handles the general case):
    dot(LN(q), LN(k)) = aq*ak*(q.k - D*muq*muk)      a = rsqrt(var+eps)
    sum_f w_f*LN(v_f) = sum_f c_f*v_f - (sum_f c_f*muv_f)*ones,  c_f = w_f*av_f
so no normalized tensor is ever materialized: only raw dots + per-token stats.

v2: all-bf16 device I/O (inputs cast on host; rel_err ~5e-3 << 2e-2 gate),
halving HBM traffic 40MiB -> 20MiB per core, plus engine rebalancing:
  - DVE: k bn_stats, dots (STT w/ accum), v-sum reduce (per V_MODE)
  - ACT: q stats (2-pass accum) or v-sq, out = psum + bias(-d)
  - Pool: small [128,4] weight math + diag builds (no STT/reduce on Pool:
    this walrus rejects TensorScalarPtr and X-axis reduce on Pool)
  - PE : bf16 matmuls: psum = ident@q + sum_f diag(c_f)@v_f

Sharding: 8 cores = batch (4) x query-half (2). Each core owns 1024 windows:
q[1024,1024], k/v[1024,4,1024] (window-major view), out[1024,1024].
"""

import numpy as np


def _ensure_concourse():
    try:
        import concourse.bass  # noqa: F401
    except ImportError:
        import sys

        for p in ("/opt/trn_rl_repo", "/root/.axon_site/_ro/trn_rl_repo"):
            if p not in sys.path:
                sys.path.insert(0, p)


_ensure_concourse()

import concourse.bass as bass  # noqa: E402
import concourse.tile as tile  # noqa: E402
from concourse import mybir  # noqa: E402
from concourse.bass_utils import run_bass_kernel_spmd  # noqa: E402

# ---------------------------------------------------------------------------
# Walrus-compatibility shims.
#
# The walrus in this container rejects two things Tile's end-of-context tail
# emits: (a) the final Drain carrying >2 sem waits ("Too many sync wait
# commands"), and (b) EVENT_SEMAPHORE_RANGE_CLEAR ("ISA wrong length").
# Replace the tail with per-semaphore EventSemaphore instructions that wait
# for each sem's final value, then the normal all-engine barrier. A JSON-level
# pass additionally splits any instruction carrying more than MAX_WAITS sem
# waits into EventSemaphore wait carriers.
# ---------------------------------------------------------------------------

_MAX_WAITS = 1


def _patched_drain_and_barrier(self, tick_clock, wait_clock):
    nc = self.nc
    gc = tick_clock.global_clock
    sems = self.sems.allocated()  # proc idx -> SemaphoreHandle
    for proc in sorted(sems):
        h = sems[proc]
        if "DMA" not in h.name:
            continue  # engine sems are implied by stream completion
        final = int(gc[proc]) * 16
        if final > 0:
            nc.gpsimd.wait_ge(h, final)
    nc.all_engine_barrier()
    popped = nc._tile_sem_poison_stack.pop()
    assert popped is self._sem_poison


tile.TileContext._drain_and_barrier = _patched_drain_and_barrier

_orig_to_json_bytes = bass.Bass.to_json_bytes


def _to_json_bytes_compat(self):
    import orjson

    raw = _orig_to_json_bytes(self)
    d = orjson.loads(raw)
    changed = False
    for fn in d.get("functions", []):
        blocks = fn.get("basic_blocks") or fn.get("blocks") or []
        for bb in blocks:
            insts = bb.get("instructions", [])
            new_insts = []
            for inst in insts:
                waits = (inst.get("sync_info") or {}).get("on_wait") or []
                if len(waits) > _MAX_WAITS:
                    keep = waits[-_MAX_WAITS:]
                    excess = waits[:-_MAX_WAITS]
                    for i, wt in enumerate(excess):
                        new_insts.append(
                            {
                                "name": f"{inst['name']}_wsplit{i}",
                                "opcode": "EventSemaphore",
                                "engine": inst["engine"],
                                "ins": [],
                                "outs": [],
                                "debug": inst.get("debug"),
                                "sync_info": {"on_update": [], "on_wait": [wt]},
                            }
                        )
                    inst["sync_info"]["on_wait"] = keep
                    changed = True
                new_insts.append(inst)
            bb["instructions"] = new_insts
    return orjson.dumps(d) if changed else raw


bass.Bass.to_json_bytes = _to_json_bytes_compat

F32 = mybir.dt.float32
BF16 = mybir.dt.bfloat16
ALU = mybir.AluOpType
ACTF = mybir.ActivationFunctionType
AXL = mybir.AxisListType

B, SQ, SKV, D = 4, 2048, 8192, 1024
FACTOR = 4
N_CORES = 8
W_PER_CORE = B * SQ // N_CORES  # 1024 windows per core
P = 128  # windows per tile = SBUF partitions
LN_EPS = 1e-5
HALF = 512  # PSUM bank free-dim (f32)

# ---- engine-assignment tunables -------------------------------------------
STAT_DT = F32  # bn_stats/bn_aggr stats dtype (f32: S-trick needs exact Sk)
QK_ENGINE = "dve"  # engine for the (q+k) adds: "dve" (TT bf16 2x) | "pool"
QK_ONE_OP = False  # (q+k) adds as one broadcast op vs 4 ops
V_DMA = "act"  # v tile DMA ring: "act" (ACT HWDGE) | "sp"


def build_bass(n_tiles=W_PER_CORE // P, repeats=1):
    """repeats>1 wraps the 8-tile body in a For_i hardware loop (timing
    NEFFs); repeats=1 is the straight-line correctness/production NEFF."""
    nc = bass.Bass()
    q_d = nc.declare_dram_parameter("q", [n_tiles * P, D], BF16, isOutput=False)
    k_d = nc.declare_dram_parameter("k", [n_tiles * P, FACTOR, D], BF16, isOutput=False)
    v_d = nc.declare_dram_parameter("v", [n_tiles * P, FACTOR, D], BF16, isOutput=False)
    id_d = nc.declare_dram_parameter("ident", [P, P], BF16, isOutput=False)
    o_d = nc.declare_dram_parameter("out", [n_tiles * P, D], BF16, isOutput=True)

    lp = nc.allow_low_precision(reason="bf16 data/accums: rel_err gate is 2e-2")
    lp.__enter__()

    with tile.TileContext(nc) as tc:
        with (
            tc.tile_pool(name="qp", bufs=3) as qp,
            tc.tile_pool(name="kp", bufs=3) as kp,
            tc.tile_pool(name="vp", bufs=3) as vp,
            tc.tile_pool(name="qkp", bufs=2) as qkp,
            tc.tile_pool(name="outp", bufs=3) as outp,
            tc.tile_pool(name="scratch", bufs=2) as scratch,
            tc.tile_pool(name="smalls", bufs=3) as sm,
            tc.tile_pool(name="const", bufs=1) as cp,
            tc.tile_pool(name="psum", bufs=3, space="PSUM") as pp,
        ):
            ident = cp.tile([P, P], BF16)
            nc.sync.dma_start(ident[:], id_d[:])

            from contextlib import nullcontext

            loop_ctx = tc.For_i(0, repeats, 1) if repeats > 1 else nullcontext()
            with loop_ctx:
                for t in range(n_tiles):
                    rows = slice(t * P, (t + 1) * P)
                    q_sb = qp.tile([P, D], BF16)
                    nc.sync.dma_start(q_sb[:], q_d[rows, :])
                    k_sb = kp.tile([P, FACTOR, D], BF16)
                    nc.sync.dma_start(k_sb[:], k_d[rows, :, :])
                    v_sb = vp.tile([P, FACTOR, D], BF16)
                    if V_DMA == "act":
                        nc.scalar.dma_start(v_sb[:], v_d[rows, :, :])
                    else:
                        nc.sync.dma_start(v_sb[:], v_d[rows, :, :])

                    # ---- q stats via ACT accumulate (2 passes)
                    sum_q = sm.tile([P, 1], F32)
                    ssq_q = sm.tile([P, 1], F32)
                    dmpq = scratch.tile([P, D], BF16, tag="actdump")
                    nc.scalar.activation(
                        dmpq[:], q_sb[:], ACTF.Copy, accum_out=sum_q[:]
                    )
                    dmpq2 = scratch.tile([P, D], BF16, tag="actdump")
                    nc.scalar.activation(
                        dmpq2[:], q_sb[:], ACTF.Square, accum_out=ssq_q[:]
                    )
                    mu_q = sm.tile([P, 1], F32)
                    nc.gpsimd.tensor_scalar_mul(mu_q[:], sum_q[:], 1.0 / D)
                    mmq = sm.tile([P, 1], F32)
                    nc.gpsimd.tensor_mul(mmq[:], mu_q[:], mu_q[:])

                    # ---- k and v stats via DVE bn_stats (fused mean+var)
                    bnst = sm.tile([P, 2, FACTOR, 2, 6], STAT_DT)  # (k/v, f, ch, 6)
                    stats = sm.tile([P, 2, FACTOR, 2], F32)  # (k/v, f, mean/var)
                    for i, x_sb in ((0, k_sb), (1, v_sb)):
                        for f in range(FACTOR):
                            for ch in range(2):
                                nc.vector.bn_stats(
                                    bnst[:, i, f, ch],
                                    x_sb[:, f, ch * HALF : (ch + 1) * HALF],
                                )
                            nc.vector.bn_aggr(
                                stats[:, i, f],
                                bnst[:, i, f].rearrange("p c x -> p (c x)"),
                            )
                    mu_k = stats[:, 0, :, 0]
                    mu_v = stats[:, 1, :, 0]

                    # ---- batched rstd for (varq, vark[4], varv[4]) = [P,9]
                    var_all = sm.tile([P, 9], F32)
                    nc.gpsimd.tensor_scalar(
                        var_all[:, 0:1], ssq_q[:], 1.0 / D, mmq[:, 0:1],
                        ALU.mult, ALU.subtract,
                    )
                    nc.gpsimd.tensor_copy(var_all[:, 1:5], stats[:, 0, :, 1])
                    nc.gpsimd.tensor_copy(var_all[:, 5:9], stats[:, 1, :, 1])
                    veps = sm.tile([P, 9], F32)
                    nc.gpsimd.tensor_scalar_add(veps[:], var_all[:], LN_EPS)
                    rall = sm.tile([P, 9], F32)
                    nc.vector.reciprocal(rall[:], veps[:])
                    a_all = sm.tile([P, 9], F32)
                    nc.scalar.sqrt(a_all[:], rall[:])
                    aq = a_all[:, 0:1]
                    ak = a_all[:, 1:5]
                    av = a_all[:, 5:9]

                    # ---- dots via S-trick: r_f = (S_f - Sq - Sk_f)/2
                    # qk_f = q + k_f on Pool (bf16), S_f = sum(qk^2) on ACT
                    qk = qkp.tile([P, FACTOR, D], BF16)
                    qk_eng = nc.vector if QK_ENGINE == "dve" else nc.gpsimd
                    if QK_ONE_OP:
                        q_b = q_sb[:].rearrange("p (f d) -> p f d", f=1)
                        qk_eng.tensor_tensor(
                            qk[:], k_sb[:], q_b.to_broadcast([P, FACTOR, D]), ALU.add
                        )
                    else:
                        for f in range(FACTOR):
                            qk_eng.tensor_tensor(
                                qk[:, f], k_sb[:, f], q_sb[:], ALU.add
                            )
                    S = sm.tile([P, FACTOR], F32)
                    for f in range(FACTOR):
                        dmps = scratch.tile([P, D], BF16, tag="actdump")
                        nc.scalar.activation(
                            dmps[:], qk[:, f], ACTF.Square,
                            accum_out=S[:, f : f + 1],
                        )
                    # Sk_f = D*(vark + muk^2); rdot = 0.5*S - 0.5*Sq - 0.5*Sk
                    mmk = sm.tile([P, FACTOR], F32)
                    nc.gpsimd.tensor_mul(mmk[:], mu_k, mu_k)
                    vpm = sm.tile([P, FACTOR], F32)
                    nc.gpsimd.tensor_tensor(vpm[:], stats[:, 0, :, 1], mmk[:], ALU.add)
                    # rdot = 0.5*(S - ssq_q) - 0.5*D*vpm
                    t_a = sm.tile([P, FACTOR], F32)
                    nc.gpsimd.tensor_scalar(
                        t_a[:], S[:], ssq_q[:, 0:1], 0.5, ALU.subtract, ALU.mult
                    )
                    rdot = sm.tile([P, FACTOR], F32)
                    nc.vector.scalar_tensor_tensor(
                        rdot[:], vpm[:], -0.5 * D, t_a[:], ALU.mult, ALU.add
                    )

                    # ---- w_f = aq*ak_f*(rdot_f - D*muq*muk_f); c_f = w_f*av_f
                    t1 = sm.tile([P, FACTOR], F32)
                    nc.gpsimd.tensor_scalar(
                        t1[:], mu_k, mu_q[:, 0:1], None, ALU.mult
                    )
                    t2 = sm.tile([P, FACTOR], F32)
                    nc.vector.scalar_tensor_tensor(
                        t2[:], t1[:], -float(D), rdot[:], ALU.mult, ALU.add
                    )
                    u = sm.tile([P, FACTOR], F32)
                    nc.gpsimd.tensor_scalar(u[:], ak, aq, None, ALU.mult)
                    w = sm.tile([P, FACTOR], F32)
                    nc.gpsimd.tensor_mul(w[:], t2[:], u[:])
                    c = sm.tile([P, FACTOR], F32)
                    nc.gpsimd.tensor_mul(c[:], w[:], av)
                    e = sm.tile([P, FACTOR], F32)
                    nc.gpsimd.tensor_mul(e[:], c[:], mu_v)
                    neg_d = sm.tile([P, 1], F32)
                    nc.vector.tensor_reduce(neg_d[:], e[:], AXL.X, ALU.add, negate=True)

                    # ---- diag(c_f) on Pool
                    diags = []
                    for f in range(FACTOR):
                        dg = sm.tile([P, P], BF16, tag=f"diag{f}")
                        nc.gpsimd.tensor_scalar_mul(dg[:], ident[:], c[:, f : f + 1])
                        diags.append(dg)

                    # ---- PE: psum[s,:] = q[s,:] + sum_f c_f[s]*v_f[s,:]
                    psum_t = pp.tile([P, 2, HALF], F32)
                    for h in range(2):
                        nc.tensor.matmul(
                            psum_t[:, h],
                            ident[:],
                            q_sb[:, h * HALF : (h + 1) * HALF],
                            start=True,
                            stop=False,
                        )
                    for f in range(FACTOR):
                        for h in range(2):
                            nc.tensor.matmul(
                                psum_t[:, h],
                                diags[f][:],
                                v_sb[:, f, h * HALF : (h + 1) * HALF],
                                start=False,
                                stop=(f == FACTOR - 1),
                            )

                    # ---- out = psum + (-d) on ACT
                    out_sb = outp.tile([P, D], BF16)
                    nc.scalar.activation(
                        out_sb[:],
                        psum_t[:].rearrange("p c x -> p (c x)"),
                        ACTF.Identity,
                        bias=neg_d[:],
                    )
                    nc.sync.dma_start(o_d[rows, :], out_sb[:])
    return nc


def make_in_map(q_core, k_core, v_core):
    """Host-side per-core input prep shared by run()/test/sim: cast to bf16."""
    import ml_dtypes

    bf = ml_dtypes.bfloat16
    return {
        "q": np.ascontiguousarray(np.asarray(q_core, dtype=np.float32)).astype(bf),
        "k": np.ascontiguousarray(np.asarray(k_core, dtype=np.float32)).astype(bf),
        "v": np.ascontiguousarray(np.asarray(v_core, dtype=np.float32)).astype(bf),
        "ident": np.eye(P, dtype=np.float32).astype(bf),
    }


_NC_CACHE = None


def _get_nc():
    global _NC_CACHE
    if _NC_CACHE is None:
        _NC_CACHE = build_bass()
    return _NC_CACHE


def _numpy_reference(query, key, value, ln_w, ln_b):
    def ln(x):
        mu = x.mean(-1, keepdims=True)
        var = ((x - mu) ** 2).mean(-1, keepdims=True)
        return (x - mu) / np.sqrt(var + LN_EPS) * ln_w + ln_b

    qn, kn, vn = ln(query), ln(key), ln(value)
    b, s, d = key.shape
    k_win = kn.reshape(b, s // FACTOR, FACTOR, d)
    wts = np.einsum("bsd,bsfd->bsf", qn, k_win).reshape(b, s)
    attn = (wts[:, :, None] * vn).reshape(b, s // FACTOR, FACTOR, d).sum(axis=2)
    return (query + attn).astype(np.float32)


def run(inputs, trace=False):
    """Returns (full_output, BassKernelResults-or-None)."""
    query = np.asarray(inputs["query"], dtype=np.float32)
    key = np.asarray(inputs["key"], dtype=np.float32)
    value = np.asarray(inputs["value"], dtype=np.float32)
    ln_w = np.asarray(inputs["ln_weight"], dtype=np.float32)
    ln_b = np.asarray(inputs["ln_bias"], dtype=np.float32)

    if not (np.all(ln_w == 1.0) and np.all(ln_b == 0.0)):
        # General-path fallback (setup_inputs always produces ones/zeros).
        return _numpy_reference(query, key, value, ln_w, ln_b), None

    sq_h = SQ // 2  # 1024 query rows per core
    skv_h = SKV // 2  # 4096 kv rows per core
    in_maps = []
    for cidx in range(N_CORES):
        bi, h = divmod(cidx, 2)
        in_maps.append(
            make_in_map(
                query[bi, h * sq_h : (h + 1) * sq_h],
                key[bi, h * skv_h : (h + 1) * skv_h].reshape(W_PER_CORE, FACTOR, D),
                value[bi, h * skv_h : (h + 1) * skv_h].reshape(W_PER_CORE, FACTOR, D),
            )
        )

    res = run_bass_kernel_spmd(
        _get_nc(), in_maps, core_ids=list(range(N_CORES)), trace=trace
    )
    out = np.empty((B, SQ, D), dtype=np.float32)
    for cidx in range(N_CORES):
        bi, h = divmod(cidx, 2)
        out[bi, h * sq_h : (h + 1) * sq_h] = np.asarray(
            res.results[cidx]["out"], dtype=np.float32
        )
    return out, res


def kernel(**inputs) -> np.ndarray:
    out, _ = run(inputs)
    return out



# revision 4
# speedup vs baseline: 1.5201x; 1.5201x over previous
"""Bass/Tile Trainium2 kernel for nn_AttentionSampling.

Problem: out = q + attention_downsampling(LN(q), LN(k), LN(v), factor=4)
  B=4, Sq=2048, Skv=8192, D=1024. Per query token s:
    w_f   = dot(LN(q)[s], LN(k)[4s+f])          f in 0..3  (no softmax)
    out[s] = q[s] + sum_f w_f * LN(v)[4s+f]

Key algebraic folding (valid for ln_weight==1, ln_bias==0, which is what
setup_inputs produces; a numpy fallback handles the general case):
    dot(LN(q), LN(k)) = aq*ak*(q.k - D*muq*muk)      a = rsqrt(var+eps)
    sum_f w_f*LN(v_f) = sum_f c_f*v_f - (sum_f c_f*muv_f)*ones,  c_f = w_f*av_f
so no normalized tensor is ever materialized: only raw dots + per-token stats.

v2: all-bf16 device I/O (inputs cast on host; rel_err ~5e-3 << 2e-2 gate),
halving HBM traffic 40MiB -> 20MiB per core, plus engine rebalancing:
  - DVE: k bn_stats, dots (STT w/ accum), v-sum reduce (per V_MODE)
  - ACT: q stats (2-pass accum) or v-sq, out = psum + bias(-d)
  - Pool: small [128,4] weight math + diag builds (no STT/reduce on Pool:
    this walrus rejects TensorScalarPtr and X-axis reduce on Pool)
  - PE : bf16 matmuls: psum = ident@q + sum_f diag(c_f)@v_f

Sharding: 8 cores = batch (4) x query-half (2). Each core owns 1024 windows:
q[1024,1024], k/v[1024,4,1024] (window-major view), out[1024,1024].
"""

import numpy as np


def _ensure_concourse():
    try:
        import concourse.bass  # noqa: F401
    except ImportError:
        import sys

        for p in ("/opt/trn_rl_repo", "/root/.axon_site/_ro/trn_rl_repo"):
            if p not in sys.path:
                sys.path.insert(0, p)


_ensure_concourse()

import concourse.bass as bass  # noqa: E402
import concourse.tile as tile  # noqa: E402
from concourse import mybir  # noqa: E402
from concourse.bass_utils import run_bass_kernel_spmd  # noqa: E402

# ---------------------------------------------------------------------------
# Walrus-compatibility shims.
#
# The walrus in this container rejects two things Tile's end-of-context tail
# emits: (a) the final Drain carrying >2 sem waits ("Too many sync wait
# commands"), and (b) EVENT_SEMAPHORE_RANGE_CLEAR ("ISA wrong length").
# Replace the tail with per-semaphore EventSemaphore instructions that wait
# for each sem's final value, then the normal all-engine barrier. A JSON-level
# pass additionally splits any instruction carrying more than MAX_WAITS sem
# waits into EventSemaphore wait carriers.
# ---------------------------------------------------------------------------

_MAX_WAITS = 1


def _patched_drain_and_barrier(self, tick_clock, wait_clock):
    nc = self.nc
    gc = tick_clock.global_clock
    sems = self.sems.allocated()  # proc idx -> SemaphoreHandle
    for proc in sorted(sems):
        h = sems[proc]
        if "DMA" not in h.name:
            continue  # engine sems are implied by stream completion
        final = int(gc[proc]) * 16
        if final > 0:
            nc.gpsimd.wait_ge(h, final)
    nc.all_engine_barrier()
    popped = nc._tile_sem_poison_stack.pop()
    assert popped is self._sem_poison


tile.TileContext._drain_and_barrier = _patched_drain_and_barrier

_orig_to_json_bytes = bass.Bass.to_json_bytes


def _to_json_bytes_compat(self):
    import orjson

    raw = _orig_to_json_bytes(self)
    d = orjson.loads(raw)
    changed = False
    for fn in d.get("functions", []):
        blocks = fn.get("basic_blocks") or fn.get("blocks") or []
        for bb in blocks:
            insts = bb.get("instructions", [])
            new_insts = []
            for inst in insts:
                waits = (inst.get("sync_info") or {}).get("on_wait") or []
                if len(waits) > _MAX_WAITS:
                    keep = waits[-_MAX_WAITS:]
                    excess = waits[:-_MAX_WAITS]
                    for i, wt in enumerate(excess):
                        new_insts.append(
                            {
                                "name": f"{inst['name']}_wsplit{i}",
                                "opcode": "EventSemaphore",
                                "engine": inst["engine"],
                                "ins": [],
                                "outs": [],
                                "debug": inst.get("debug"),
                                "sync_info": {"on_update": [], "on_wait": [wt]},
                            }
                        )
                    inst["sync_info"]["on_wait"] = keep
                    changed = True
                new_insts.append(inst)
            bb["instructions"] = new_insts
    return orjson.dumps(d) if changed else raw


bass.Bass.to_json_bytes = _to_json_bytes_compat

F32 = mybir.dt.float32
BF16 = mybir.dt.bfloat16
ALU = mybir.AluOpType
ACTF = mybir.ActivationFunctionType
AXL = mybir.AxisListType

B, SQ, SKV, D = 4, 2048, 8192, 1024
FACTOR = 4
N_CORES = 8
W_PER_CORE = B * SQ // N_CORES  # 1024 windows per core
P = 128  # windows per tile = SBUF partitions
LN_EPS = 1e-5
HALF = 512  # PSUM bank free-dim (f32)

# ---- engine-assignment tunables -------------------------------------------
STAT_DT = F32  # bn_stats/bn_aggr stats dtype (f32: S-trick needs exact Sk)
V_DMA = "act"  # v tile DMA ring: "act" (ACT HWDGE) | "sp"
QK_ON_PE = True  # build q+k_f in PSUM via I@q + I@k_f (frees DVE adds)


def build_bass(n_tiles=W_PER_CORE // P, repeats=1):
    """repeats>1 wraps the 8-tile body in a For_i hardware loop (timing
    NEFFs); repeats=1 is the straight-line correctness/production NEFF."""
    nc = bass.Bass()
    q_d = nc.declare_dram_parameter("q", [n_tiles * P, D], BF16, isOutput=False)
    k_d = nc.declare_dram_parameter("k", [n_tiles * P, FACTOR, D], BF16, isOutput=False)
    v_d = nc.declare_dram_parameter("v", [n_tiles * P, FACTOR, D], BF16, isOutput=False)
    id_d = nc.declare_dram_parameter("ident", [P, P], BF16, isOutput=False)
    o_d = nc.declare_dram_parameter("out", [n_tiles * P, D], BF16, isOutput=True)

    lp = nc.allow_low_precision(reason="bf16 data/accums: rel_err gate is 2e-2")
    lp.__enter__()

    with tile.TileContext(nc) as tc:
        with (
            tc.tile_pool(name="qp", bufs=3) as qp,
            tc.tile_pool(name="kp", bufs=3) as kp,
            tc.tile_pool(name="vp", bufs=3) as vp,
            tc.tile_pool(name="outp", bufs=3) as outp,
            tc.tile_pool(name="scratch", bufs=2) as scratch,
            tc.tile_pool(name="smalls", bufs=3) as sm,
            tc.tile_pool(name="const", bufs=1) as cp,
            tc.tile_pool(name="psum", bufs=2, space="PSUM") as pp,
            tc.tile_pool(name="qkpsum", bufs=2, space="PSUM") as qkpp,
        ):
            ident = cp.tile([P, P], BF16)
            nc.sync.dma_start(ident[:], id_d[:])

            from contextlib import nullcontext

            loop_ctx = tc.For_i(0, repeats, 1) if repeats > 1 else nullcontext()
            with loop_ctx:
                for t in range(n_tiles):
                    rows = slice(t * P, (t + 1) * P)
                    q_sb = qp.tile([P, D], BF16)
                    nc.sync.dma_start(q_sb[:], q_d[rows, :])
                    k_sb = kp.tile([P, FACTOR, D], BF16)
                    nc.sync.dma_start(k_sb[:], k_d[rows, :, :])
                    v_sb = vp.tile([P, FACTOR, D], BF16)
                    if V_DMA == "act":
                        nc.scalar.dma_start(v_sb[:], v_d[rows, :, :])
                    else:
                        nc.sync.dma_start(v_sb[:], v_d[rows, :, :])

                    # ---- q stats via ACT accumulate (2 passes)
                    sum_q = sm.tile([P, 1], F32)
                    ssq_q = sm.tile([P, 1], F32)
                    dmpq = scratch.tile([P, D], BF16, tag="actdump")
                    nc.scalar.activation(
                        dmpq[:], q_sb[:], ACTF.Copy, accum_out=sum_q[:]
                    )
                    dmpq2 = scratch.tile([P, D], BF16, tag="actdump")
                    nc.scalar.activation(
                        dmpq2[:], q_sb[:], ACTF.Square, accum_out=ssq_q[:]
                    )
                    mu_q = sm.tile([P, 1], F32)
                    nc.gpsimd.tensor_scalar_mul(mu_q[:], sum_q[:], 1.0 / D)
                    mmq = sm.tile([P, 1], F32)
                    nc.gpsimd.tensor_mul(mmq[:], mu_q[:], mu_q[:])

                    # ---- k and v stats via DVE bn_stats (fused mean+var)
                    bnst = sm.tile([P, 2, FACTOR, 2, 6], STAT_DT)  # (k/v, f, ch, 6)
                    stats = sm.tile([P, 2, FACTOR, 2], F32)  # (k/v, f, mean/var)
                    for i, x_sb in ((0, k_sb), (1, v_sb)):
                        for f in range(FACTOR):
                            for ch in range(2):
                                nc.vector.bn_stats(
                                    bnst[:, i, f, ch],
                                    x_sb[:, f, ch * HALF : (ch + 1) * HALF],
                                )
                            nc.vector.bn_aggr(
                                stats[:, i, f],
                                bnst[:, i, f].rearrange("p c x -> p (c x)"),
                            )
                    mu_k = stats[:, 0, :, 0]
                    mu_v = stats[:, 1, :, 0]

                    # ---- batched rstd for (varq, vark[4], varv[4]) = [P,9]
                    var_all = sm.tile([P, 9], F32)
                    nc.gpsimd.tensor_scalar(
                        var_all[:, 0:1], ssq_q[:], 1.0 / D, mmq[:, 0:1],
                        ALU.mult, ALU.subtract,
                    )
                    nc.gpsimd.tensor_copy(var_all[:, 1:5], stats[:, 0, :, 1])
                    nc.gpsimd.tensor_copy(var_all[:, 5:9], stats[:, 1, :, 1])
                    veps = sm.tile([P, 9], F32)
                    nc.gpsimd.tensor_scalar_add(veps[:], var_all[:], LN_EPS)
                    rall = sm.tile([P, 9], F32)
                    nc.vector.reciprocal(rall[:], veps[:])
                    a_all = sm.tile([P, 9], F32)
                    nc.scalar.sqrt(a_all[:], rall[:])
                    aq = a_all[:, 0:1]
                    ak = a_all[:, 1:5]
                    av = a_all[:, 5:9]

                    # ---- dots via S-trick: r_f = (S_f - Sq - Sk_f)/2
                    # qk_f = q + k_f built on PE (psum = I@q + I@k_f), then
                    # S_f = sum(qk^2) on ACT reading PSUM. Frees the DVE adds.
                    S = sm.tile([P, FACTOR], F32)
                    for f in range(FACTOR):
                        qk_ps = qkpp.tile([P, 2, HALF], F32, tag="qk")
                        for h in range(2):
                            nc.tensor.matmul(
                                qk_ps[:, h],
                                ident[:],
                                q_sb[:, h * HALF : (h + 1) * HALF],
                                start=True,
                                stop=False,
                            )
                            nc.tensor.matmul(
                                qk_ps[:, h],
                                ident[:],
                                k_sb[:, f, h * HALF : (h + 1) * HALF],
                                start=False,
                                stop=True,
                            )
                        dmps = scratch.tile([P, D], BF16, tag="actdump")
                        nc.scalar.activation(
                            dmps[:],
                            qk_ps[:].rearrange("p c x -> p (c x)"),
                            ACTF.Square,
                            accum_out=S[:, f : f + 1],
                        )
                    # Sk_f = D*(vark + muk^2); rdot = 0.5*S - 0.5*Sq - 0.5*Sk
                    mmk = sm.tile([P, FACTOR], F32)
                    nc.gpsimd.tensor_mul(mmk[:], mu_k, mu_k)
                    vpm = sm.tile([P, FACTOR], F32)
                    nc.gpsimd.tensor_tensor(vpm[:], stats[:, 0, :, 1], mmk[:], ALU.add)
                    # rdot = 0.5*(S - ssq_q) - 0.5*D*vpm
                    t_a = sm.tile([P, FACTOR], F32)
                    nc.gpsimd.tensor_scalar(
                        t_a[:], S[:], ssq_q[:, 0:1], 0.5, ALU.subtract, ALU.mult
                    )
                    rdot = sm.tile([P, FACTOR], F32)
                    nc.vector.scalar_tensor_tensor(
                        rdot[:], vpm[:], -0.5 * D, t_a[:], ALU.mult, ALU.add
                    )

                    # ---- w_f = aq*ak_f*(rdot_f - D*muq*muk_f); c_f = w_f*av_f
                    t1 = sm.tile([P, FACTOR], F32)
                    nc.gpsimd.tensor_scalar(
                        t1[:], mu_k, mu_q[:, 0:1], None, ALU.mult
                    )
                    t2 = sm.tile([P, FACTOR], F32)
                    nc.vector.scalar_tensor_tensor(
                        t2[:], t1[:], -float(D), rdot[:], ALU.mult, ALU.add
                    )
                    u = sm.tile([P, FACTOR], F32)
                    nc.gpsimd.tensor_scalar(u[:], ak, aq, None, ALU.mult)
                    w = sm.tile([P, FACTOR], F32)
                    nc.gpsimd.tensor_mul(w[:], t2[:], u[:])
                    c = sm.tile([P, FACTOR], F32)
                    nc.gpsimd.tensor_mul(c[:], w[:], av)
                    e = sm.tile([P, FACTOR], F32)
                    nc.gpsimd.tensor_mul(e[:], c[:], mu_v)
                    neg_d = sm.tile([P, 1], F32)
                    nc.vector.tensor_reduce(neg_d[:], e[:], AXL.X, ALU.add, negate=True)

                    # ---- diag(c_f) on Pool
                    diags = []
                    for f in range(FACTOR):
                        dg = sm.tile([P, P], BF16, tag=f"diag{f}")
                        nc.gpsimd.tensor_scalar_mul(dg[:], ident[:], c[:, f : f + 1])
                        diags.append(dg)

                    # ---- PE: psum[s,:] = q[s,:] + sum_f c_f[s]*v_f[s,:]
                    psum_t = pp.tile([P, 2, HALF], F32)
                    for h in range(2):
                        nc.tensor.matmul(
                            psum_t[:, h],
                            ident[:],
                            q_sb[:, h * HALF : (h + 1) * HALF],
                            start=True,
                            stop=False,
                        )
                    for f in range(FACTOR):
                        for h in range(2):
                            nc.tensor.matmul(
                                psum_t[:, h],
                                diags[f][:],
                                v_sb[:, f, h * HALF : (h + 1) * HALF],
                                start=False,
                                stop=(f == FACTOR - 1),
                            )

                    # ---- out = psum + (-d) on ACT
                    out_sb = outp.tile([P, D], BF16)
                    nc.scalar.activation(
                        out_sb[:],
                        psum_t[:].rearrange("p c x -> p (c x)"),
                        ACTF.Identity,
                        bias=neg_d[:],
                    )
                    nc.sync.dma_start(o_d[rows, :], out_sb[:])
    return nc


def make_in_map(q_core, k_core, v_core):
    """Host-side per-core input prep shared by run()/test/sim: cast to bf16."""
    import ml_dtypes

    bf = ml_dtypes.bfloat16
    return {
        "q": np.ascontiguousarray(np.asarray(q_core, dtype=np.float32)).astype(bf),
        "k": np.ascontiguousarray(np.asarray(k_core, dtype=np.float32)).astype(bf),
        "v": np.ascontiguousarray(np.asarray(v_core, dtype=np.float32)).astype(bf),
        "ident": np.eye(P, dtype=np.float32).astype(bf),
    }


_NC_CACHE = None


def _get_nc():
    global _NC_CACHE
    if _NC_CACHE is None:
        _NC_CACHE = build_bass()
    return _NC_CACHE


def _numpy_reference(query, key, value, ln_w, ln_b):
    def ln(x):
        mu = x.mean(-1, keepdims=True)
        var = ((x - mu) ** 2).mean(-1, keepdims=True)
        return (x - mu) / np.sqrt(var + LN_EPS) * ln_w + ln_b

    qn, kn, vn = ln(query), ln(key), ln(value)
    b, s, d = key.shape
    k_win = kn.reshape(b, s // FACTOR, FACTOR, d)
    wts = np.einsum("bsd,bsfd->bsf", qn, k_win).reshape(b, s)
    attn = (wts[:, :, None] * vn).reshape(b, s // FACTOR, FACTOR, d).sum(axis=2)
    return (query + attn).astype(np.float32)


def run(inputs, trace=False):
    """Returns (full_output, BassKernelResults-or-None)."""
    query = np.asarray(inputs["query"], dtype=np.float32)
    key = np.asarray(inputs["key"], dtype=np.float32)
    value = np.asarray(inputs["value"], dtype=np.float32)
    ln_w = np.asarray(inputs["ln_weight"], dtype=np.float32)
    ln_b = np.asarray(inputs["ln_bias"], dtype=np.float32)

    if not (np.all(ln_w == 1.0) and np.all(ln_b == 0.0)):
        # General-path fallback (setup_inputs always produces ones/zeros).
        return _numpy_reference(query, key, value, ln_w, ln_b), None

    sq_h = SQ // 2  # 1024 query rows per core
    skv_h = SKV // 2  # 4096 kv rows per core
    in_maps = []
    for cidx in range(N_CORES):
        bi, h = divmod(cidx, 2)
        in_maps.append(
            make_in_map(
                query[bi, h * sq_h : (h + 1) * sq_h],
                key[bi, h * skv_h : (h + 1) * skv_h].reshape(W_PER_CORE, FACTOR, D),
                value[bi, h * skv_h : (h + 1) * skv_h].reshape(W_PER_CORE, FACTOR, D),
            )
        )

    res = run_bass_kernel_spmd(
        _get_nc(), in_maps, core_ids=list(range(N_CORES)), trace=trace
    )
    out = np.empty((B, SQ, D), dtype=np.float32)
    for cidx in range(N_CORES):
        bi, h = divmod(cidx, 2)
        out[bi, h * sq_h : (h + 1) * sq_h] = np.asarray(
            res.results[cidx]["out"], dtype=np.float32
        )
    return out, res


def kernel(**inputs) -> np.ndarray:
    out, _ = run(inputs)
    return out

